# revision 16
# baseline (speedup 1.0000x reference)
"""MDN-RNN mixture-density loss kernel for Trainium2, SPMD over 8 NeuronCores.

Math (per token row i):
    means/logstds: [K, D] slices of s_mean/s_logstd rows
    z      = (target - mean_k) * exp(-logstd_k)
    logp_k = -0.5 * sum_d z^2 - sum_d logstd_k
    loss   = -mean_i logsumexp_k(log_mix_coeffs + logp_k)

Sharding: data-parallel on the token dim N=16384 -> R=2048 rows per core;
each core emits a [128,1] partial sum of per-row -logsumexp values,
combined into the scalar mean on the host.

Layout: the host TRANSPOSES the per-core shards to [D, K, R] (d on
partitions) and casts to bf16. This makes both free-dim reductions of the
row-major layout (sum_d logstd, sum_d z^2) PARTITION reductions, which the
otherwise-idle TensorEngine computes as ones-vector matmuls into PSUM:

    psum_s[k, r] += sum_d -1.0 * logstd[d, k, r]    (sel_s stationary)
    psum_h[k, r] += sum_d -0.5 * z^2[d, k, r]       (sel_h stationary)

so  score[k, r] = lmx[k, r] + psum_s + psum_h  needs only two adds.

Engine split per d-chunk (8 full chunks of 128 + one of 64):
    ACT:  e1 = exp(-logstd) (one [P, K*R] instr), z^2 for k in SQ_ACT_KS
    DVE:  diff = tgt - mean (bf16 TT 2x), z = diff*e1 (2x), z^2 for SQ_DVE_KS
    GPS:  z^2 for SQ_GPS_KS
    PE:   40 ones-matmuls (5 k x 4 psum windows x {sls, h})

Tail: scores [5, R] merge (2 TT), PE-transpose to [128, 16, 5], then a
sub-microsecond logsumexp over k=5 and the final row reduction.
"""

import sys

if "/opt/trn_rl_repo" not in sys.path:
    sys.path.insert(0, "/opt/trn_rl_repo")

import numpy as np
import ml_dtypes

N = 16384
K = 5
D = 1088
NCORES = 8
R = N // NCORES          # 2048 rows per core
P = 128                  # partitions
T = R // P               # 16 row-groups per core (tail layout)
CP = [128] * 8 + [64]    # partitions per d-chunk
NCH = len(CP)
WN = 512                 # psum window cols (one bank of fp32)
NW = R // WN             # 4 windows

# z^2 engine split by k
SQ_DVE_KS = (0,)
SQ_ACT_KS = (2, 3)
SQ_GPS_KS = (1, 4)
MUL_GPS_KS = (4,)        # z = diff*e1 k-slices on GPSIMD; rest on DVE

_NC = None


def _build():
    import concourse.bacc as bacc
    import concourse.bass as bass
    import concourse.tile as tile
    from concourse import mybir

    AF = mybir.ActivationFunctionType
    AL = mybir.AluOpType
    AX = mybir.AxisListType
    f32 = mybir.dt.float32
    bf16 = mybir.dt.bfloat16
    f8 = mybir.dt.float8e4

    nc = bacc.Bacc("TRN2", debug=False)
    tgtT = nc.dram_tensor("tgtT", [D, R], bf16, kind="ExternalInput").ap()
    meanT = nc.dram_tensor("meanT", [D, K * R], bf16, kind="ExternalInput").ap()
    lstdT = nc.dram_tensor("lstdT", [D, K * R], f8, kind="ExternalInput").ap()
    lmxT = nc.dram_tensor("lmxT", [K, R], f32, kind="ExternalInput").ap()
    # selh[:, k, :] = -0.5 at col k (h matmuls, bf16 rhs)
    # sels[:, k, :] = -1.0 at col k (sls matmuls, fp8 rhs)
    selh = nc.dram_tensor("selh", [P, K * K], bf16, kind="ExternalInput").ap()
    sels = nc.dram_tensor("sels", [P, K * K], f8, kind="ExternalInput").ap()
    id5 = nc.dram_tensor("id5", [K, K], f32, kind="ExternalInput").ap()
    out = nc.dram_tensor("partial", [P, 1], f32, kind="ExternalOutput").ap()

    with tile.TileContext(nc) as tc:
        with (
            tc.tile_pool(name="tgt_p", bufs=3) as tgt_p,
            tc.tile_pool(name="mean_p", bufs=3) as mean_p,
            tc.tile_pool(name="lstd_p", bufs=3) as lstd_p,
            tc.tile_pool(name="e1_p", bufs=2) as e1_p,
            tc.tile_pool(name="small_p", bufs=2) as small_p,
            tc.tile_pool(name="persist", bufs=1) as persist,
            tc.tile_pool(name="ps_h", bufs=1, space="PSUM") as ps_h,
            tc.tile_pool(name="ps_s", bufs=1, space="PSUM") as ps_s,
        ):
            t_selh = persist.tile([P, K, K], bf16)
            nc.sync.dma_start(out=t_selh, in_=selh.rearrange("p (k m) -> p k m", k=K))
            t_sels = persist.tile([P, K, K], f8)
            nc.sync.dma_start(out=t_sels, in_=sels.rearrange("p (k m) -> p k m", k=K))
            t_id5 = persist.tile([K, K], f32)
            nc.sync.dma_start(out=t_id5, in_=id5)
            t_lmx = persist.tile([K, R], f32)
            nc.sync.dma_start(out=t_lmx, in_=lmxT)

            t_ph = ps_h.tile([K, R], f32)   # -0.5 * sum_d z^2, per (k, row)
            t_ps = ps_s.tile([K, R], f32)   # -1.0 * sum_d logstd

            state = {}

            def emit_a(c):
                """DMAs, e1 = exp(-logstd), sls matmuls (need lstd only)."""
                cp = CP[c]
                drows = slice(c * P, c * P + cp)
                t_tgt = tgt_p.tile([P, R], bf16)
                t_mean = mean_p.tile([P, K, R], bf16)
                t_lstd = lstd_p.tile([P, K, R], f8)
                mean3 = meanT[drows].rearrange("p (k r) -> p k r", k=K)
                lstd3 = lstdT[drows].rearrange("p (k r) -> p k r", k=K)
                if c > 0:
                    nc.sync.dma_start(out=t_lstd[:cp], in_=lstd3)
                    nc.sync.dma_start(out=t_tgt[:cp], in_=tgtT[drows])
                    nc.sync.dma_start(out=t_mean[:cp], in_=mean3)
                else:
                    # chunked so first compute starts after ~1/5 of the load
                    for k in range(K):
                        nc.sync.dma_start(out=t_lstd[:cp, k, :], in_=lstd3[:, k, :])
                    nc.sync.dma_start(out=t_tgt[:cp], in_=tgtT[drows])
                    for k in range(K):
                        nc.sync.dma_start(out=t_mean[:cp, k, :], in_=mean3[:, k, :])

                t_e1 = e1_p.tile([P, K, R], bf16)
                nc.scalar.activation(
                    out=t_e1[:cp], in_=t_lstd[:cp], func=AF.Exp, scale=-1.0
                )
                # psum_s += sel_s[k]^T @ lstd[:, k, w]
                for k in range(K):
                    for w in range(NW):
                        nc.tensor.matmul(
                            t_ps[:, w * WN : (w + 1) * WN],
                            lhsT=t_sels[:cp, k, :],
                            rhs=t_lstd[:cp, k, w * WN : (w + 1) * WN],
                            start=(c == 0 and k == 0),
                            stop=(c == NCH - 1 and k == K - 1),
                        )
                state[c] = (t_tgt, t_mean, t_e1)

            def emit_b(c):
                """diff (DVE, bf16 2x) and z (DVE + GPS split)."""
                cp = CP[c]
                t_tgt, t_mean, t_e1 = state[c]
                tgt_b = bass.AP(
                    tensor=t_tgt.tensor, offset=t_tgt.offset,
                    ap=[[t_tgt.ap[0][0], cp], [0, K], t_tgt.ap[1]],
                )
                nc.vector.tensor_tensor(
                    out=t_mean[:cp], in0=tgt_b, in1=t_mean[:cp], op=AL.subtract
                )
                kd = K - len(MUL_GPS_KS)  # DVE takes k < kd (contiguous 3D slice)
                nc.vector.tensor_tensor(
                    out=t_mean[:cp, :kd], in0=t_mean[:cp, :kd],
                    in1=t_e1[:cp, :kd], op=AL.mult,
                )
                for k in MUL_GPS_KS:
                    nc.gpsimd.tensor_tensor(
                        out=t_mean[:cp, k, :], in0=t_mean[:cp, k, :],
                        in1=t_e1[:cp, k, :], op=AL.mult,
                    )

            def emit_c(c):
                """z^2 in place (3-engine split), then h matmuls."""
                cp = CP[c]
                t_tgt, t_mean, t_e1 = state.pop(c)
                for k in range(K):
                    zk = t_mean[:cp, k, :]
                    if k in SQ_DVE_KS:
                        nc.vector.tensor_tensor(out=zk, in0=zk, in1=zk, op=AL.mult)
                    elif k in SQ_ACT_KS:
                        nc.scalar.activation(out=zk, in_=zk, func=AF.Square)
                    else:
                        nc.gpsimd.tensor_tensor(out=zk, in0=zk, in1=zk, op=AL.mult)
                for k in range(K):
                    for w in range(NW):
                        nc.tensor.matmul(
                            t_ph[:, w * WN : (w + 1) * WN],
                            lhsT=t_selh[:cp, k, :],
                            rhs=t_mean[:cp, k, w * WN : (w + 1) * WN],
                            start=(c == 0 and k == 0),
                            stop=(c == NCH - 1 and k == K - 1),
                        )

            emit_a(0)
            emit_a(1)
            emit_b(0)
            for c in range(NCH):
                emit_c(c)
                if c + 1 < NCH:
                    emit_b(c + 1)
                if c + 2 < NCH:
                    emit_a(c + 2)

            # ---- tail: score merge, transpose to [128, T, K], logsumexp ----
            t_hs = persist.tile([K, R], f32)
            nc.vector.tensor_tensor(out=t_hs, in0=t_ph, in1=t_lmx, op=AL.add)
            nc.vector.tensor_tensor(out=t_hs, in0=t_hs, in1=t_ps, op=AL.add)

            # PE transpose [K, 128]-blocks -> psum scores [128, T, K].
            # Reuses t_ph's banks (t_ph is dead after the merge above).
            t_tc = ps_h.tile([P, T, K], f32, tag="t_ph")
            for t in range(T):
                nc.tensor.transpose(
                    t_tc[:, t, :], t_hs[:, t * P : (t + 1) * P], t_id5
                )

            t_nmx = persist.tile([P, T], f32)
            nc.vector.tensor_reduce(
                out=t_nmx, in_=t_tc, axis=AX.X, op=AL.max, negate=True
            )
            nmx_b = bass.AP(
                tensor=t_nmx.tensor, offset=t_nmx.offset,
                ap=[t_nmx.ap[0], t_nmx.ap[1], [0, K]],
            )
            t_cm = persist.tile([P, T, K], f32)
            nc.vector.tensor_tensor(out=t_cm, in0=t_tc, in1=nmx_b, op=AL.add)
            t_e = persist.tile([P, T, K], f32)
            nc.scalar.activation(out=t_e, in_=t_cm, func=AF.Exp)
            t_S = persist.tile([P, T], f32)
            nc.vector.tensor_reduce(out=t_S, in_=t_e, axis=AX.X, op=AL.add)
            t_lns = persist.tile([P, T], f32)
            nc.scalar.activation(out=t_lns, in_=t_S, func=AF.Ln)
            # logsumexp = lnS - nmx; partial = -sum_t logsumexp
            t_lr = persist.tile([P, T], f32)
            nc.vector.tensor_tensor(out=t_lr, in0=t_lns, in1=t_nmx, op=AL.subtract)
            t_tot = persist.tile([P, 1], f32)
            nc.vector.tensor_reduce(
                out=t_tot, in_=t_lr, axis=AX.X, op=AL.add, negate=True
            )
            nc.sync.dma_start(out=out, in_=t_tot)

    nc.compile()
    return nc


def get_nc():
    global _NC
    if _NC is None:
        _NC = _build()
    return _NC


def make_in_maps(target, s_mean, s_logstd, log_mix_coeffs):
    bf = ml_dtypes.bfloat16
    f8 = ml_dtypes.float8_e4m3fn
    target = np.asarray(target, dtype=np.float32).astype(bf)
    s_mean = np.asarray(s_mean, dtype=np.float32).astype(bf)
    s_logstd = np.asarray(s_logstd, dtype=np.float32).astype(f8)
    lm = np.asarray(log_mix_coeffs, dtype=np.float32)

    selh = np.zeros((P, K, K), dtype=bf)
    sels = np.zeros((P, K, K), dtype=f8)
    for k in range(K):
        selh[:, k, k] = bf(-0.5)
        sels[:, k, k] = f8(-1.0)
    selh = np.ascontiguousarray(selh.reshape(P, K * K))
    sels = np.ascontiguousarray(sels.reshape(P, K * K))
    id5 = np.ascontiguousarray(np.eye(K, dtype=np.float32))

    in_maps = []
    for c in range(NCORES):
        rows = slice(c * R, (c + 1) * R)
        tT = np.ascontiguousarray(target[rows].T)                     # [D, R]
        mT = np.ascontiguousarray(
            s_mean[rows].reshape(R, K, D).transpose(2, 1, 0).reshape(D, K * R)
        )
        lT = np.ascontiguousarray(
            s_logstd[rows].reshape(R, K, D).transpose(2, 1, 0).reshape(D, K * R)
        )
        lmxT = np.ascontiguousarray(lm[rows].T)                       # [K, R]
        in_maps.append({
            "tgtT": tT, "meanT": mT, "lstdT": lT,
            "lmxT": lmxT, "selh": selh, "sels": sels, "id5": id5,
        })
    return in_maps


def combine(results):
    total = sum(float(np.asarray(r["partial"], dtype=np.float64).sum()) for r in results)
    return np.float32(total / N)


def kernel(target, s_mean, s_logstd, log_mix_coeffs):
    from concourse.bass_utils import run_bass_kernel_spmd

    nc = get_nc()
    in_maps = make_in_maps(target, s_mean, s_logstd, log_mix_coeffs)
    res = run_bass_kernel_spmd(nc, in_maps, core_ids=list(range(NCORES)))
    return combine(res.results)
